# revision 3
# baseline (speedup 1.0000x reference)
"""MBTFEConv (Chebyshev heat-kernel multi-band GNN) Trainium2 kernel, 8 cores.

Self-contained: hardcodes the problem shapes (N=50000, E=800000, IN=OUT=128,
K=8, M=3 bands) and the node-sharding strategy. Accepts FULL inputs, returns
the FULL [N, OUT] output.

Strategy
--------
Nodes are row-sharded across 8 cores (padded to 50176 = 8*6272 rows, 49
128-row tiles per core). Edges are grouped by destination tile; each core owns
its destination tiles' edges. Per Chebyshev step:
  - per destination tile, gather source-node rows Psi_{k-1}[col] (fp16) from
    the allgathered full Psi via SWDGE dma_gather (indices are host-
    precomputed int16, split into lo/hi half-tables for the int16 range),
  - build the one-hot-times-weight selection matrix SelT[e,d] = w_e*(dest_e==d)
    on the vector engine (broadcast is_equal, then mult),
  - segment-sum via fp16 TensorE matmuls accumulating in fp32 PSUM:
    psum[d,f] = sum_e SelT[e,d] * C[e,f],
  - Chebyshev combine 2*psum - Psi_{k-2} in fp32 (one DVE op), keeping the
    fp32 master shard in SBUF,
  - write the fp16-cast shard tile to the AllGather input, and the PE-
    transposed (feature-major) fp16 tile to a local DRAM stack for the
    epilogue,
  - AllGather (8 cores) re-assembles the full fp16 Psi_k for the next step's
    gathers and doubles as the cross-core barrier.
The band/fuse epilogue folds the heat-kernel band mixing into the band linear:
W'[i,k] = B[i,k] * W_band[i], so H_i^T = relu(sum_k W'[i,k]^T @ Psi_k^T + b),
then OUT^T = W_fuse^T @ concat(H_0..H_3, X)^T + b_fuse, all feature-major on
the PE; the host transposes the per-core [128, 6272] outputs back.
"""

import math

import numpy as np

import concourse.bacc as bacc
import concourse.tile as tile
from concourse import bass, mybir
from concourse.bass_utils import run_bass_kernel_spmd
from concourse.masks import make_identity

# ---- problem constants (hardcoded per contract) ----
N = 50000
E = 800000
F = 128            # in_dim == out_dim
KCH = 8            # Chebyshev order
TAUS = [0.5, 1.5, 4.0]
MB = 4             # M+1 bands
NCORES = 8
P = 128
NT = 49            # dest tiles per core
PER_CORE = NT * P  # 6272
NPAD = NCORES * PER_CORE  # 50176
HALF = NPAD // 2   # 25088 (int16 index range split)
ST_TILES = 2       # dest tiles per supertile (gather batching)
EPIL_GW = 512      # epilogue node-group width
GMAX_CH = 8        # max 128-chunks per dma_gather (SWDGE ring: 1024 descs)


def _iv(k, tau, terms=60):
    return sum(
        (tau / 2.0) ** (2 * m + k) / (math.factorial(m) * math.factorial(m + k))
        for m in range(terms)
    )


def _bessel_coeffs_heat(tau, K):
    if tau == 0.0:
        a = np.zeros(K + 1)
        a[0] = 1.0
        return a
    ks = np.arange(K + 1)
    Ik = np.array([_iv(int(k), tau) for k in ks])
    e = math.exp(-tau)
    a = 2.0 * e * ((-1.0) ** ks) * Ik
    a[0] = e * Ik[0]
    return a


A_COEF = np.stack([_bessel_coeffs_heat(t, KCH) for t in [0.0] + TAUS]).astype(
    np.float64
)  # [4, 9]
B_COEF = np.concatenate([A_COEF[:-1] - A_COEF[1:], A_COEF[-1:]], axis=0)  # [4, 9]

_CACHE = {}


def _wrap_idx(lin):
    """int16 linear index list (len % 128 == 0) -> [128, len//16] wrapped SBUF
    layout: idxs[p, s] = lin[s*16 + p] for p<16, replicated to 128 partitions."""
    n = len(lin)
    w = np.zeros((16, n // 16), dtype=np.int16)
    w[np.arange(n) % 16, np.arange(n) // 16] = lin
    return np.tile(w, (8, 1))


def _preprocess(edge_row, edge_col, edge_weight):
    """Group edges by (core, dest-tile-slot, src-half), sort by col, pad to
    uniform per-slot chunk counts across cores. Returns per-core gather index
    / dest / weight arrays plus the (shared) chunk structure."""
    row = np.asarray(edge_row).astype(np.int64)
    col = np.asarray(edge_col).astype(np.int64)
    w = np.asarray(edge_weight).astype(np.float64)

    tile_id = row // P                 # 0..390
    core = tile_id // NT
    slot = tile_id % NT
    half = (col >= HALF).astype(np.int64)

    # order: (core, slot, half, col)  -> contiguous segments
    order = np.lexsort((col, half, slot, core))
    row, col, w, core, slot, half = (
        row[order], col[order], w[order], core[order], slot[order], half[order]
    )
    dest_local = (row % P).astype(np.float64)

    # segment counts [NCORES, NT, 2]
    cnt = np.zeros((NCORES, NT, 2), dtype=np.int64)
    np.add.at(cnt, (core, slot, half), 1)
    nchunk = -(-cnt // P)  # ceil
    NCL = nchunk[:, :, 0].max(axis=0)  # [NT] uniform lo chunk counts
    NCH2 = nchunk[:, :, 1].max(axis=0)

    # supertiles
    sts = [list(range(i, min(i + ST_TILES, NT))) for i in range(0, NT, ST_TILES)]

    # per-(core,slot,half) edge segment start offsets in the sorted arrays
    seg_start = np.zeros((NCORES, NT, 2), dtype=np.int64)
    flat_cnt = cnt.reshape(-1)
    seg_start.reshape(-1)[1:] = np.cumsum(flat_cnt)[:-1]

    # global chunk layout: for each supertile: [lo chunks of members..., hi...]
    # build per-core arrays
    NCHTOT = int(NCL.sum() + NCH2.sum())
    idx_cols = NCHTOT * P // 16
    idx_all = np.zeros((NCORES, 128, idx_cols), dtype=np.int16)
    dest_all = np.zeros((NCORES, 128, NCHTOT), dtype=np.float16)
    w_all = np.zeros((NCORES, 128, NCHTOT), dtype=np.float16)

    # structure info for the program: per supertile:
    #   (gather segs [(idx_col_off, nidx, half)], chunk->slot list, chunk base)
    st_info = []
    q = 0          # global chunk cursor
    icol = 0       # idx_all column cursor
    for members in sts:
        ncl = [int(NCL[s]) for s in members]
        nch = [int(NCH2[s]) for s in members]
        n_lo = sum(ncl) * P
        n_hi = sum(nch) * P
        chunk_slots = []
        for s, n in zip(members, ncl):
            chunk_slots += [s] * n
        for s, n in zip(members, nch):
            chunk_slots += [s] * n
        st_info.append(
            dict(
                members=members,
                q0=q,
                nct=len(chunk_slots),
                chunk_slots=chunk_slots,
                lo=(icol, n_lo),
                hi=(icol + n_lo // 16, n_hi),
            )
        )
        for c in range(NCORES):
            lin_lo = np.zeros(n_lo, dtype=np.int64)
            lin_hi = np.zeros(n_hi, dtype=np.int64)
            d_st = np.zeros((len(chunk_slots), P), dtype=np.float64)
            w_st = np.zeros((len(chunk_slots), P), dtype=np.float64)
            lo_pos = 0
            qq = 0
            for s, n in zip(members, ncl):
                a = seg_start[c, s, 0]
                m = cnt[c, s, 0]
                lin_lo[lo_pos:lo_pos + m] = col[a:a + m]
                dd = np.zeros(n * P)
                ww = np.zeros(n * P)
                dd[:m] = dest_local[a:a + m]
                ww[:m] = w[a:a + m]
                d_st[qq:qq + n] = dd.reshape(n, P)
                w_st[qq:qq + n] = ww.reshape(n, P)
                lo_pos += n * P
                qq += n
            hi_pos = 0
            for s, n in zip(members, nch):
                a = seg_start[c, s, 1]
                m = cnt[c, s, 1]
                lin_hi[hi_pos:hi_pos + m] = col[a:a + m] - HALF
                dd = np.zeros(n * P)
                ww = np.zeros(n * P)
                dd[:m] = dest_local[a:a + m]
                ww[:m] = w[a:a + m]
                d_st[qq:qq + n] = dd.reshape(n, P)
                w_st[qq:qq + n] = ww.reshape(n, P)
                hi_pos += n * P
                qq += n
            if n_lo:
                idx_all[c][:, st_info[-1]["lo"][0]:st_info[-1]["lo"][0] + n_lo // 16] = \
                    _wrap_idx(lin_lo.astype(np.int16))
            if n_hi:
                idx_all[c][:, st_info[-1]["hi"][0]:st_info[-1]["hi"][0] + n_hi // 16] = \
                    _wrap_idx(lin_hi.astype(np.int16))
            # dest/w arrays: [p, q] = edge (q_local*128+p)
            dest_all[c][:, q:q + len(chunk_slots)] = d_st.T.astype(np.float16)
            w_all[c][:, q:q + len(chunk_slots)] = w_st.T.astype(np.float16)
        q += len(chunk_slots)
        icol += (n_lo + n_hi) // 16

    assert q == NCHTOT and icol == idx_cols
    return dict(
        idx_all=idx_all, dest_all=dest_all, w_all=w_all,
        st_info=st_info, NCHTOT=NCHTOT, idx_cols=idx_cols,
    )


def _build(st_info, NCHTOT, idx_cols):
    """Build the (single, SPMD) Bass program."""
    nc = bacc.Bacc("TRN2", target_bir_lowering=False, debug=False,
                   num_devices=NCORES)
    f16, f32, i16 = mybir.dt.float16, mybir.dt.float32, mybir.dt.int16

    t_x16 = nc.dram_tensor("t_x16", [NPAD, F], f16, kind="ExternalInput")
    t_xshard = nc.dram_tensor("t_xshard", [PER_CORE, F], f32, kind="ExternalInput")
    t_xshardT = nc.dram_tensor("t_xshardT", [P, PER_CORE], f16, kind="ExternalInput")
    t_idx = nc.dram_tensor("t_idx", [128, idx_cols], i16, kind="ExternalInput")
    t_dest = nc.dram_tensor("t_dest", [128, NCHTOT], f16, kind="ExternalInput")
    t_w = nc.dram_tensor("t_w", [128, NCHTOT], f16, kind="ExternalInput")
    t_iota = nc.dram_tensor("t_iota", [P, P], f16, kind="ExternalInput")
    t_wp = nc.dram_tensor("t_wp", [MB * 9, F, F], f16, kind="ExternalInput")
    t_wf = nc.dram_tensor("t_wf", [5, F, F], f16, kind="ExternalInput")
    t_bband = nc.dram_tensor("t_bband", [P, MB], f32, kind="ExternalInput")
    t_bfuse = nc.dram_tensor("t_bfuse", [P, 1], f32, kind="ExternalInput")
    t_out = nc.dram_tensor("t_out", [P, PER_CORE], f32, kind="ExternalOutput")

    # local DRAM stack of feature-major fp16 Psi_k^T shards (k=1..8)
    t_psiT = [
        nc.dram_tensor(f"t_psiT{k}", [P, PER_CORE], f16)
        for k in range(1, KCH + 1)
    ]

    with tile.TileContext(nc) as tc:
        with (
            tc.tile_pool(name="sb", bufs=1) as sb,
            tc.tile_pool(name="sbw", bufs=1) as sbw,
            tc.tile_pool(name="ps_pool", bufs=2, space="PSUM") as psp,
            tc.tile_pool(name="dram", bufs=1, space="DRAM") as dram,
        ):
            # ---- persistent SBUF state ----
            idx_sb = sb.tile([128, idx_cols], i16)
            dest_sb = sb.tile([128, NCHTOT], f16)
            w_sb = sb.tile([128, NCHTOT], f16)
            iota_sb = sb.tile([P, P], f16)
            ident = sb.tile([P, P], f32)
            nc.sync.dma_start(out=idx_sb[:], in_=t_idx[:, :])
            nc.sync.dma_start(out=dest_sb[:], in_=t_dest[:, :])
            nc.sync.dma_start(out=w_sb[:], in_=t_w[:, :])
            nc.sync.dma_start(out=iota_sb[:], in_=t_iota[:, :])
            make_identity(nc, ident[:])

            # fp32 node-major Psi shards, 3 rotating slots [128, 49*128]
            psi_nm = []

            def new_psi():
                t = sb.tile([P, PER_CORE], f32, tag="psiNM", bufs=3,
                            name=f"psiNM_{len(psi_nm)}")
                psi_nm.append(t)
                return t

            psi0 = new_psi()
            nc.sync.dma_start(
                out=psi0[:].rearrange("p (t f) -> p t f", t=NT),
                in_=t_xshard[:, :].rearrange("(t p) f -> p t f", p=P),
            )

            # AG ping-pong buffers
            ag_in = [
                dram.tile([PER_CORE, F], f16, tag="ag_in", bufs=2,
                          name=f"ag_in_{k}")
                for k in range(KCH - 1)
            ]
            psi_full = [
                dram.tile([NPAD, F], f16, tag="psi_full", bufs=2,
                          addr_space="Shared", name=f"psi_full_{k}")
                for k in range(KCH - 1)
            ]

            # ---- Chebyshev steps ----
            for k in range(1, KCH + 1):
                cur = new_psi()
                prev2 = psi_nm[k - 2]
                if k == 1:
                    src = t_x16[:, :]
                else:
                    src = psi_full[k - 2][:]
                src_lo = src[0:HALF, :]
                src_hi = src[HALF:NPAD, :]

                for st in st_info:
                    nct = st["nct"]
                    ct = sb.tile([128, nct * F], f16, tag="ct", bufs=3,
                                 name=f"ct_{k}_{st['q0']}")
                    io_lo, n_lo = st["lo"]
                    io_hi, n_hi = st["hi"]
                    ncl_tot = n_lo // P
                    for base_ch, n_ch, io, srcv in (
                        (0, ncl_tot, io_lo, src_lo),
                        (ncl_tot, nct - ncl_tot, io_hi, src_hi),
                    ):
                        a = 0
                        while a < n_ch:
                            b = min(a + GMAX_CH, n_ch)
                            nidx = (b - a) * P
                            nc.gpsimd.dma_gather(
                                ct[:, (base_ch + a) * F:(base_ch + b) * F]
                                .rearrange("p (c f) -> p c f", c=b - a),
                                srcv,
                                idx_sb[:, io + a * 8:io + b * 8],
                                nidx, nidx, F,
                            )
                            a = b
                    # SelT = (dest == iota) * w    [128, nct*128] fp16
                    selt = sb.tile([128, nct * P], f16, tag="selt", bufs=2,
                                   name=f"selt_{k}_{st['q0']}")
                    selt3 = selt[:].rearrange("p (c d) -> p c d", c=nct)
                    q0 = st["q0"]
                    dest3 = dest_sb[:, q0:q0 + nct].rearrange(
                        "p (c o) -> p c o", c=nct).to_broadcast([128, nct, P])
                    w3 = w_sb[:, q0:q0 + nct].rearrange(
                        "p (c o) -> p c o", c=nct).to_broadcast([128, nct, P])
                    iota3 = iota_sb[:].rearrange(
                        "p (o d) -> p o d", o=1).to_broadcast([128, nct, P])
                    nc.vector.tensor_tensor(
                        out=selt3, in0=dest3, in1=iota3,
                        op=mybir.AluOpType.is_equal)
                    nc.vector.tensor_tensor(
                        out=selt3, in0=selt3, in1=w3, op=mybir.AluOpType.mult)

                    # per member tile: accumulate psum over its chunks
                    for s in st["members"]:
                        chunks = [j for j, cs in enumerate(st["chunk_slots"])
                                  if cs == s]
                        ps = psp.tile([P, F], f32, tag="ps", bufs=2,
                                      name=f"ps_{k}_{s}")
                        for jj, j in enumerate(chunks):
                            nc.tensor.matmul(
                                out=ps[:],
                                lhsT=selt[:, j * P:(j + 1) * P],
                                rhs=ct[:, j * F:(j + 1) * F],
                                start=(jj == 0),
                                stop=(jj == len(chunks) - 1),
                            )
                        cur_sl = cur[:, s * P:(s + 1) * P]
                        if k == 1:
                            nc.vector.tensor_copy(out=cur_sl, in_=ps[:])
                        else:
                            nc.vector.scalar_tensor_tensor(
                                out=cur_sl, in0=ps[:], scalar=2.0,
                                in1=prev2[:, s * P:(s + 1) * P],
                                op0=mybir.AluOpType.mult,
                                op1=mybir.AluOpType.subtract,
                            )
                        # fp16 node-major copy -> AG input rows
                        if k < KCH:
                            tn16 = sb.tile([P, P], f16, tag="tn16", bufs=2,
                                           name=f"tn16_{k}_{s}")
                            nc.vector.tensor_copy(out=tn16[:], in_=cur_sl)
                            nc.sync.dma_start(
                                out=ag_in[k - 1][s * P:(s + 1) * P, :],
                                in_=tn16[:])
                        # feature-major fp16 -> psiT stack
                        pst = psp.tile([P, P], f32, tag="pst", bufs=2,
                                       name=f"pst_{k}_{s}")
                        nc.tensor.transpose(out=pst[:], in_=cur_sl,
                                            identity=ident[:])
                        tt16 = sb.tile([P, P], f16, tag="tt16", bufs=2,
                                       name=f"tt16_{k}_{s}")
                        nc.vector.tensor_copy(out=tt16[:], in_=pst[:])
                        nc.sync.dma_start(
                            out=t_psiT[k - 1][:, s * P:(s + 1) * P],
                            in_=tt16[:])

                if k < KCH:
                    nc.gpsimd.collective_compute(
                        "AllGather",
                        mybir.AluOpType.bypass,
                        replica_groups=[list(range(NCORES))],
                        ins=[ag_in[k - 1][:].opt()],
                        outs=[psi_full[k - 1][:].opt()],
                    )

            # ---- epilogue: H bands + fuse, feature-major ----
            wp_sb = sbw.tile([P, MB * 9 * F], f16)
            wf_sb = sbw.tile([P, 5 * F], f16)
            bband_sb = sbw.tile([P, MB], f32)
            bfuse_sb = sbw.tile([P, 1], f32)
            nc.sync.dma_start(
                out=wp_sb[:].rearrange("p (k o) -> p k o", k=MB * 9),
                in_=t_wp[:, :, :].rearrange("k f o -> f k o"))
            nc.sync.dma_start(
                out=wf_sb[:].rearrange("p (k o) -> p k o", k=5),
                in_=t_wf[:, :, :].rearrange("k f o -> f k o"))
            nc.sync.dma_start(out=bband_sb[:], in_=t_bband[:, :])
            nc.sync.dma_start(out=bfuse_sb[:], in_=t_bfuse[:, :])

            for g0 in range(0, PER_CORE, EPIL_GW):
                gw = min(EPIL_GW, PER_CORE - g0)
                pts = []
                for k in range(KCH + 1):
                    pt = sb.tile([P, gw], f16, tag=f"pt{k}", bufs=2,
                                 name=f"pt_{k}_{g0}", padded_shape=[P, EPIL_GW])
                    if k == 0:
                        nc.sync.dma_start(out=pt[:],
                                          in_=t_xshardT[:, g0:g0 + gw])
                    else:
                        nc.sync.dma_start(out=pt[:],
                                          in_=t_psiT[k - 1][:, g0:g0 + gw])
                    pts.append(pt)
                hs = []
                for i in range(MB):
                    hps = psp.tile([P, gw], f32, tag="hps", bufs=2,
                                   name=f"hps_{i}_{g0}",
                                   padded_shape=[P, EPIL_GW])
                    for k in range(KCH + 1):
                        wslice = wp_sb[:, (i * 9 + k) * F:(i * 9 + k + 1) * F]
                        nc.tensor.matmul(out=hps[:], lhsT=wslice, rhs=pts[k][:],
                                         start=(k == 0), stop=(k == KCH))
                    h = sb.tile([P, gw], f16, tag=f"h{i}", bufs=2,
                                name=f"h_{i}_{g0}", padded_shape=[P, EPIL_GW])
                    nc.vector.tensor_scalar(
                        out=h[:], in0=hps[:], scalar1=bband_sb[:, i:i + 1],
                        scalar2=0.0, op0=mybir.AluOpType.add,
                        op1=mybir.AluOpType.max)
                    hs.append(h)
                fps = psp.tile([P, gw], f32, tag="fps", bufs=2,
                               name=f"fps_{g0}", padded_shape=[P, EPIL_GW])
                rhss = [hs[0], hs[1], hs[2], hs[3], pts[0]]
                for u, r in enumerate(rhss):
                    nc.tensor.matmul(out=fps[:], lhsT=wf_sb[:, u * F:(u + 1) * F],
                                     rhs=r[:], start=(u == 0), stop=(u == 4))
                ot = sb.tile([P, gw], f32, tag="ot", bufs=2, name=f"ot_{g0}",
                             padded_shape=[P, EPIL_GW])
                nc.vector.tensor_scalar(
                    out=ot[:], in0=fps[:], scalar1=bfuse_sb[:, 0:1],
                    scalar2=None, op0=mybir.AluOpType.add)
                nc.sync.dma_start(out=t_out[:, g0:g0 + gw], in_=ot[:])

    nc.compile()
    return nc


def _make_in_maps(inputs, prep):
    X = np.asarray(inputs["X"], dtype=np.float32)
    W_band = np.asarray(inputs["W_band"], dtype=np.float32)
    b_band = np.asarray(inputs["b_band"], dtype=np.float32)
    W_fuse = np.asarray(inputs["W_fuse"], dtype=np.float32)
    b_fuse = np.asarray(inputs["b_fuse"], dtype=np.float32)

    xpad = np.zeros((NPAD, F), dtype=np.float32)
    xpad[:N] = X
    x16 = xpad.astype(np.float16)
    iota = np.tile(np.arange(P, dtype=np.float16)[None, :], (P, 1))
    wp = np.zeros((MB * 9, F, F), dtype=np.float16)
    for i in range(MB):
        for k in range(KCH + 1):
            wp[i * 9 + k] = (B_COEF[i, k] * W_band[i].astype(np.float64)).astype(
                np.float16)
    wf = np.zeros((5, F, F), dtype=np.float16)
    for u in range(5):
        wf[u] = W_fuse[u * F:(u + 1) * F].astype(np.float16)
    bband = np.zeros((P, MB), dtype=np.float32)
    bband[:, :] = b_band.T
    bfuse = b_fuse.reshape(P, 1).astype(np.float32)

    in_maps = []
    for c in range(NCORES):
        sl = slice(c * PER_CORE, (c + 1) * PER_CORE)
        in_maps.append({
            "t_x16": x16,
            "t_xshard": np.ascontiguousarray(xpad[sl]),
            "t_xshardT": np.ascontiguousarray(xpad[sl].T.astype(np.float16)),
            "t_idx": prep["idx_all"][c],
            "t_dest": prep["dest_all"][c],
            "t_w": prep["w_all"][c],
            "t_iota": iota,
            "t_wp": wp,
            "t_wf": wf,
            "t_bband": bband,
            "t_bfuse": bfuse,
        })
    return in_maps


def kernel(**inputs) -> np.ndarray:
    prep = _preprocess(inputs["edge_row"], inputs["edge_col"],
                       inputs["edge_weight"])
    key = tuple(
        (tuple(st["members"]), st["q0"], st["nct"], st["lo"], st["hi"],
         tuple(st["chunk_slots"]))
        for st in prep["st_info"]
    )
    if key not in _CACHE:
        _CACHE[key] = _build(prep["st_info"], prep["NCHTOT"], prep["idx_cols"])
    nc = _CACHE[key]

    in_maps = _make_in_maps(inputs, prep)
    res = run_bass_kernel_spmd(nc, in_maps, core_ids=list(range(NCORES)))

    out = np.empty((NPAD, F), dtype=np.float32)
    for c in range(NCORES):
        out[c * PER_CORE:(c + 1) * PER_CORE] = res.results[c]["t_out"].T
    return out[:N]


# expose for test harness reuse
def _run_traced(inputs, trace=False, **kw):
    prep = _preprocess(inputs["edge_row"], inputs["edge_col"],
                       inputs["edge_weight"])
    key = tuple(
        (tuple(st["members"]), st["q0"], st["nct"], st["lo"], st["hi"],
         tuple(st["chunk_slots"]))
        for st in prep["st_info"]
    )
    if key not in _CACHE:
        _CACHE[key] = _build(prep["st_info"], prep["NCHTOT"], prep["idx_cols"])
    nc = _CACHE[key]
    in_maps = _make_in_maps(inputs, prep)
    return run_bass_kernel_spmd(nc, in_maps, core_ids=list(range(NCORES)),
                                trace=trace, **kw)
